# revision 24
# baseline (speedup 1.0000x reference)
"""Masked multi-head attention (B=2, S=2048, E=1024, H=16, D=64) on 8 TRN2 cores.

Sharding: each core owns 2 heads (of 16) for BOTH batches.
  - QKV projections computed per-core for its 2 heads (transposed layouts).
  - Attention: flash-style with transposed scores (scoresT[k, q] tiles), unsafe
    softmax (no max subtraction -- scores are ~N(0,1), exp cannot overflow),
    denominator accumulated via a ones-column appended to V in the PV matmul.
  - AllToAll over all 8 cores re-shards head-parallel -> sequence-parallel.
  - Output projection row-parallel over the gathered heads, bias fused,
    each core emits a transposed [1024, 512] slice; host transposes + stacks.

Compute dtype bf16 (fp32 PSUM accumulation); emulated rel-l2 error vs the
fp32 reference is ~5e-3.
"""

import numpy as np
import ml_dtypes

BF16 = ml_dtypes.bfloat16

B, S, E, H, D = 2, 2048, 1024, 16, 64
P = 128
SG = B * S          # 4096 global sequence length (batch-major)
NKO = E // P        # 8 contraction tiles over E
NST = SG // P       # 32 seq tiles of 128
NSB = SG // 512     # 8 seq blocks of 512
QB = S // 512       # 4 q-blocks per batch

_built = None
LAST_RESULTS = None


def _build():
    global _built
    if _built is not None:
        return _built

    import concourse.bacc as bacc
    import concourse.mybir as mybir
    import concourse.tile as tile
    from concourse.bass import ds as bass_ds

    f32 = mybir.dt.float32
    bf16 = mybir.dt.bfloat16
    Exp = mybir.ActivationFunctionType.Exp
    Identity = mybir.ActivationFunctionType.Identity

    nc = bacc.Bacc("TRN2", target_bir_lowering=False, debug=False, num_devices=8)

    xT = nc.declare_dram_parameter("xT", [E, SG], bf16, isOutput=False)
    wq = nc.declare_dram_parameter("wq", [E, P], bf16, isOutput=False)
    wk = nc.declare_dram_parameter("wk", [E, P], bf16, isOutput=False)
    wv = nc.declare_dram_parameter("wv", [E, P], bf16, isOutput=False)
    wo = nc.declare_dram_parameter("wo", [E, E], bf16, isOutput=False)
    bo = nc.declare_dram_parameter("bo", [P, NKO], f32, isOutput=False)
    masks = nc.declare_dram_parameter("masks", [P, 2048], bf16, isOutput=False)
    outT = nc.declare_dram_parameter("outT", [E, 512], f32, isOutput=True)

    # per-slot exchange buffers: slot s = (batch s//4, q-block s%4).
    # ag_out[s] = AllGather of ag_in[s] across the 8 cores (rank-major).
    ag_in = nc.dram_tensor("ag_in", [8, P, 512], bf16)
    ag_out = nc.dram_tensor("ag_out", [8, 8, P, 512], bf16)

    with tile.TileContext(nc) as tc, \
         tc.tile_pool(name="const", bufs=1) as const:
        # ---- constant / persistent SBUF tensors ----
        xT_sb = const.tile([P, NKO, SG], bf16, name="xT_sb")
        wq_sb = const.tile([P, NKO, P], bf16, name="wq_sb")
        wk_sb = const.tile([P, NKO, P], bf16, name="wk_sb")
        wv_sb = const.tile([P, NKO, P], bf16, name="wv_sb")
        wo_sb = const.tile([P, NKO, E], bf16, name="wo_sb")
        bo_sb = const.tile([P, NKO], f32, name="bo_sb")
        masks_sb = const.tile([P, 2048], bf16, name="masks_sb")
        qT_sb = const.tile([P, SG], bf16, name="qT_sb")
        kT_sb = const.tile([P, SG], bf16, name="kT_sb")
        vT_sb = const.tile([P, SG], bf16, name="vT_sb")
        v_nat = const.tile([P, NST, P], bf16, name="v_nat")
        # per seq-tile: [ones | v_h0(64) | ones | v_h1(64)] -- the leading ones
        # column makes the softmax denominator land on PSUM partition 0
        v_aug = const.tile([P, NST, 130], bf16, name="v_aug")

        # chunked loads (by seq-block) so the first projection block only
        # waits on its own 1MB slice of x
        nc.sync.dma_start(wq_sb, wq.rearrange("(ko p) m -> p ko m", p=P))
        nc.sync.dma_start(wk_sb, wk.rearrange("(ko p) m -> p ko m", p=P))
        nc.sync.dma_start(wv_sb, wv.rearrange("(ko p) m -> p ko m", p=P))
        xT_r = xT.rearrange("(ko p) s -> p ko s", p=P)
        for sb in range(NSB):
            nc.sync.dma_start(
                xT_sb[:, :, sb * 512:(sb + 1) * 512],
                xT_r[:, :, sb * 512:(sb + 1) * 512],
            )
        nc.sync.dma_start(masks_sb, masks[:])
        nc.sync.dma_start(bo_sb, bo[:])
        nc.sync.dma_start(wo_sb, wo.rearrange("(ko p) m -> p ko m", p=P))

        # ---- phases A+B share one PSUM pool set (no phase barrier), and
        # batch-1 projection blocks are emitted between batch-0 attention
        # units so PE always has dense work while ACT runs exp ----
        with tc.tile_pool(name="psBig", bufs=2, space="PSUM") as psBig, \
             tc.tile_pool(name="psSmall", bufs=4, space="PSUM") as psSmall, \
             tc.tile_pool(name="sb_att", bufs=3) as sba:
            nc.any.memset(v_aug[:, :, 0:1], 1.0)
            nc.any.memset(v_aug[:, :, 65:66], 1.0)

            def proj_block(w_sb, dst, sb):
                ps = psBig.tile([P, 2, 512], f32, tag="big", name="ps_proj")
                for ko in range(NKO):
                    nc.tensor.matmul(
                        ps[:, 0, :],
                        w_sb[:, ko, :],
                        xT_sb[:, ko, sb * 512:(sb + 1) * 512],
                        start=(ko == 0),
                        stop=(ko == NKO - 1),
                    )
                nc.vector.tensor_copy(out=dst[:, sb * 512:(sb + 1) * 512], in_=ps[:, 0, :])

            def v_block(sb):
                # vT for seq-block sb (dense N=512 matmuls), then DMA-xbar
                # transpose into natural [s, dh] layout and stripe into v_aug
                proj_block(wv_sb, vT_sb, sb)
                for st in range(4 * sb, 4 * sb + 4):
                    nc.sync.dma_start_transpose(
                        v_nat[:, st, :], vT_sb[:, st * P:(st + 1) * P]
                    )
                    nc.vector.tensor_copy(
                        out=v_aug[:, st, 0:130].rearrange("p (h x) -> p h x", x=65)[:, :, 1:65],
                        in_=v_nat[:, st, :].rearrange("p (h x) -> p h x", x=64),
                    )

            def attn_unit(b, qb):
                # both local heads; score matmuls on PE row-groups 0-1 / 2-3
                # run concurrently; k-tiles in pairs -> [128, 2, 512] exp ops
                numer = [
                    psSmall.tile([65, 512], f32, tag="small", name="ps_nm_t")
                    for _ in range(2)
                ]
                nkt = 4 * qb + 4
                for kt0 in range(0, nkt, 2):
                    sc = [
                        psBig.tile([P, 2, 512], f32, tag="big", name="ps_sc_t")
                        for _ in range(2)
                    ]
                    ex = [
                        sba.tile([P, 2, 512], bf16, tag=f"exp{hl}", name="sb_ex_t")
                        for hl in range(2)
                    ]
                    for j in range(2):
                        for hl in range(2):
                            nc.tensor.matmul(
                                sc[hl][:, j, :],
                                kT_sb[64 * hl:64 * hl + 64,
                                      S * b + (kt0 + j) * P:S * b + (kt0 + j + 1) * P],
                                qT_sb[64 * hl:64 * hl + 64,
                                      S * b + qb * 512:S * b + (qb + 1) * 512],
                                start=True,
                                stop=True,
                            )
                    for hl in range(2):
                        nc.scalar.activation(ex[hl], sc[hl], Exp, scale=0.125)
                    r = kt0 - 4 * qb
                    if r >= 0:
                        mrow = masks_sb[:, r * 512:(r + 2) * 512].rearrange(
                            "p (j f) -> p j f", j=2
                        )
                        for hl in range(2):
                            nc.vector.tensor_mul(out=ex[hl], in0=ex[hl], in1=mrow)
                    for j in range(2):
                        kt = kt0 + j
                        for hl in range(2):
                            nc.tensor.matmul(
                                numer[hl],
                                v_aug[:, 16 * b + kt, 65 * hl:65 * hl + 65],
                                ex[hl][:, j, :],
                                start=(kt == 0),
                                stop=(kt == nkt - 1),
                            )
                for hl in range(2):
                    recip = sba.tile([1, 512], f32, tag="recip", name="sb_rc_t")
                    nc.vector.reciprocal_approx_fast(recip, numer[hl][0:1, :])
                    rb = sba.tile([65, 512], f32, tag="rbcast", name="sb_rb_t")
                    nc.gpsimd.partition_broadcast(rb, recip)
                    attn = sba.tile([65, 512], bf16, tag="attn", name="sb_at_t")
                    nc.vector.tensor_mul(out=attn, in0=numer[hl][:, :], in1=rb)
                    nc.sync.dma_start(
                        ag_in[4 * b + qb, 64 * hl:64 * hl + 64, :], attn[1:65, :]
                    )

            def gather_slot(s):
                # fire the per-slot exchange as soon as slot s's attention is
                # written; all but the last overlap remaining compute
                nc.gpsimd.collective_compute(
                    "AllGather",
                    mybir.AluOpType.bypass,
                    replica_groups=[list(range(8))],
                    ins=[ag_in[s].opt()],
                    outs=[ag_out[s].opt()],
                )

            # batch-0 inputs first
            for sb in range(4):
                proj_block(wq_sb, qT_sb, sb)
                proj_block(wk_sb, kT_sb, sb)
                v_block(sb)

            # batch-0 attention interleaved with batch-1 q/k projections;
            # batch-1 v-blocks emitted just ahead of their consuming units
            a1 = []
            for sb in range(4, 8):
                a1.append(lambda sb=sb: proj_block(wq_sb, qT_sb, sb))
                a1.append(lambda sb=sb: proj_block(wk_sb, kT_sb, sb))
            for qb in range(QB):
                attn_unit(0, qb)
                gather_slot(qb)
                take, a1 = a1[:2], a1[2:]
                for thunk in take:
                    thunk()
            for thunk in a1:
                thunk()
            for qb in range(QB):
                v_block(4 + qb)
                attn_unit(1, qb)
                gather_slot(4 + qb)

            # ---- phase D: output projection; this core's slice selected by
            # a partition-id-offset DMA out of its slot's gather ----
            pid = nc.sync.partition_id()
            attn_all = const.tile([P, 8, 512], bf16, name="attn_all")
            for ci in range(8):
                nc.sync.dma_start(
                    attn_all[:, ci, :],
                    ag_out[bass_ds(pid, 1), ci].rearrange("o p f -> (o p) f"),
                )
            out_sb = const.tile([P, NKO, 512], f32, name="out_sb")
            outT_r = outT.rearrange("(mo p) f -> p mo f", p=P)
            for mo in range(NKO):
                ps = psBig.tile([P, 2, 512], f32, tag="big", name="ps_out")
                for ci in range(8):
                    nc.tensor.matmul(
                        ps[:, 0, :],
                        wo_sb[:, ci, mo * P:(mo + 1) * P],
                        attn_all[:, ci, :],
                        start=(ci == 0),
                        stop=(ci == 7),
                    )
                nc.scalar.activation(
                    out_sb[:, mo, :], ps[:, 0, :], Identity,
                    bias=bo_sb[:, mo:mo + 1], scale=1.0,
                )
                nc.sync.dma_start(outT_r[:, mo:mo + 1, :], out_sb[:, mo:mo + 1, :])

    nc.compile()
    _built = nc
    return nc


def _host_masks():
    p = np.arange(P)[:, None]
    f = np.arange(512)[None, :]
    m = np.zeros((P, 4, 512), np.float32)
    for r in range(4):
        m[:, r, :] = (f >= P * r + p).astype(np.float32)
    return np.ascontiguousarray(m.reshape(P, 2048)).astype(BF16)


def kernel(**inputs):
    global LAST_RESULTS
    from concourse import bass_utils

    x = np.asarray(inputs["x"], np.float32)
    W_q = np.asarray(inputs["W_q"], np.float32)
    W_k = np.asarray(inputs["W_k"], np.float32)
    W_v = np.asarray(inputs["W_v"], np.float32)
    W_o = np.asarray(inputs["W_o"], np.float32)
    b_o = np.asarray(inputs["b_o"], np.float32)

    nc = _build()

    xT_all = np.ascontiguousarray(
        np.concatenate([x[0].T, x[1].T], axis=1)
    ).astype(BF16)
    wo_b = np.ascontiguousarray(W_o).astype(BF16)
    bo_t = np.ascontiguousarray(b_o.reshape(NKO, P).T).astype(np.float32)
    masks = _host_masks()

    in_maps = []
    for c in range(8):
        sl = slice(P * c, P * (c + 1))
        in_maps.append({
            "xT": xT_all,
            "wq": np.ascontiguousarray(W_q[:, sl]).astype(BF16),
            "wk": np.ascontiguousarray(W_k[:, sl]).astype(BF16),
            "wv": np.ascontiguousarray(W_v[:, sl]).astype(BF16),
            "wo": wo_b,
            "bo": bo_t,
            "masks": masks,
        })

    res = bass_utils.run_bass_kernel_spmd(nc, in_maps, core_ids=list(range(8)))
    LAST_RESULTS = res

    out = np.empty((B, S, E), np.float32)
    for c in range(8):
        b, qb = c // 4, c % 4
        out[b, 512 * qb:512 * (qb + 1), :] = np.asarray(
            res.results[c]["outT"], np.float32
        ).T
    return out


# revision 27
# speedup vs baseline: 1.1870x; 1.1870x over previous
"""Masked multi-head attention (B=2, S=2048, E=1024, H=16, D=64) on 8 TRN2 cores.

Sharding: each core owns 2 heads (of 16) for BOTH batches.
  - QKV projections computed per-core for its 2 heads (transposed layouts).
  - Attention: flash-style with transposed scores (scoresT[k, q] tiles), unsafe
    softmax (no max subtraction -- scores are ~N(0,1), exp cannot overflow),
    denominator accumulated via a ones-column appended to V in the PV matmul.
  - AllToAll over all 8 cores re-shards head-parallel -> sequence-parallel.
  - Output projection row-parallel over the gathered heads, bias fused,
    each core emits a transposed [1024, 512] slice; host transposes + stacks.

Compute dtype bf16 (fp32 PSUM accumulation); emulated rel-l2 error vs the
fp32 reference is ~5e-3.
"""

import numpy as np
import ml_dtypes

BF16 = ml_dtypes.bfloat16

B, S, E, H, D = 2, 2048, 1024, 16, 64
P = 128
SG = B * S          # 4096 global sequence length (batch-major)
NKO = E // P        # 8 contraction tiles over E
NST = SG // P       # 32 seq tiles of 128
NSB = SG // 512     # 8 seq blocks of 512
QB = S // 512       # 4 q-blocks per batch

_built = None
LAST_RESULTS = None


def _build():
    global _built
    if _built is not None:
        return _built

    import concourse.bacc as bacc
    import concourse.mybir as mybir
    import concourse.tile as tile
    from concourse.bass import ds as bass_ds

    f32 = mybir.dt.float32
    bf16 = mybir.dt.bfloat16
    Exp = mybir.ActivationFunctionType.Exp
    Identity = mybir.ActivationFunctionType.Identity

    nc = bacc.Bacc("TRN2", target_bir_lowering=False, debug=False, num_devices=8)

    xT = nc.declare_dram_parameter("xT", [E, SG], bf16, isOutput=False)
    wq = nc.declare_dram_parameter("wq", [E, P], bf16, isOutput=False)
    wk = nc.declare_dram_parameter("wk", [E, P], bf16, isOutput=False)
    wv = nc.declare_dram_parameter("wv", [E, P], bf16, isOutput=False)
    wo = nc.declare_dram_parameter("wo", [E, E], bf16, isOutput=False)
    bo = nc.declare_dram_parameter("bo", [P, NKO], f32, isOutput=False)
    masks = nc.declare_dram_parameter("masks", [P, 2048], bf16, isOutput=False)
    outT = nc.declare_dram_parameter("outT", [E, 512], f32, isOutput=True)

    # per-slot exchange buffers: slot s = (batch s//4, q-block s%4).
    # ag_out[s] = AllGather of ag_in[s] across the 8 cores (rank-major).
    ag_in = nc.dram_tensor("ag_in", [8, P, 512], bf16)
    ag_out = nc.dram_tensor("ag_out", [8, 8, P, 512], bf16)

    with tile.TileContext(nc) as tc, \
         tc.tile_pool(name="const", bufs=1) as const:
        # ---- constant / persistent SBUF tensors ----
        xT_sb = const.tile([P, NKO, SG], bf16, name="xT_sb")
        wq_sb = const.tile([P, NKO, P], bf16, name="wq_sb")
        wk_sb = const.tile([P, NKO, P], bf16, name="wk_sb")
        wv_sb = const.tile([P, NKO, P], bf16, name="wv_sb")
        wo_sb = const.tile([P, NKO, E], bf16, name="wo_sb")
        bo_sb = const.tile([P, NKO], f32, name="bo_sb")
        masks_sb = const.tile([P, 2048], bf16, name="masks_sb")
        qT_sb = const.tile([P, SG], bf16, name="qT_sb")
        kT_sb = const.tile([P, SG], bf16, name="kT_sb")
        # per seq-tile: [ones | v_h0(64) | ones | v_h1(64)] -- the leading ones
        # column makes the softmax denominator land on PSUM partition 0
        v_aug = const.tile([P, NST, 130], bf16, name="v_aug")

        # chunked loads (by seq-block) so the first projection block only
        # waits on its own 1MB slice of x
        nc.sync.dma_start(wq_sb, wq.rearrange("(ko p) m -> p ko m", p=P))
        nc.sync.dma_start(wk_sb, wk.rearrange("(ko p) m -> p ko m", p=P))
        nc.sync.dma_start(wv_sb, wv.rearrange("(ko p) m -> p ko m", p=P))
        xT_r = xT.rearrange("(ko p) s -> p ko s", p=P)
        for sb in range(NSB):
            nc.sync.dma_start(
                xT_sb[:, :, sb * 512:(sb + 1) * 512],
                xT_r[:, :, sb * 512:(sb + 1) * 512],
            )
        nc.sync.dma_start(masks_sb, masks[:])
        nc.sync.dma_start(bo_sb, bo[:])
        nc.sync.dma_start(wo_sb, wo.rearrange("(ko p) m -> p ko m", p=P))

        # ---- phases A+B share one PSUM pool set (no phase barrier), and
        # batch-1 projection blocks are emitted between batch-0 attention
        # units so PE always has dense work while ACT runs exp ----
        with tc.tile_pool(name="psBig", bufs=2, space="PSUM") as psBig, \
             tc.tile_pool(name="psSmall", bufs=4, space="PSUM") as psSmall, \
             tc.tile_pool(name="sb_att", bufs=3) as sba:
            nc.any.memset(v_aug[:, :, 0:1], 1.0)
            nc.any.memset(v_aug[:, :, 65:66], 1.0)

            def proj_block(w_sb, dst, sb):
                ps = psBig.tile([P, 2, 512], f32, tag="big", name="ps_proj")
                for ko in range(NKO):
                    nc.tensor.matmul(
                        ps[:, 0, :],
                        w_sb[:, ko, :],
                        xT_sb[:, ko, sb * 512:(sb + 1) * 512],
                        start=(ko == 0),
                        stop=(ko == NKO - 1),
                    )
                nc.vector.tensor_copy(out=dst[:, sb * 512:(sb + 1) * 512], in_=ps[:, 0, :])

            def v_block(st):
                ps = psSmall.tile([P, P], f32, tag="small", name="ps_vproj")
                for ko in range(NKO):
                    nc.tensor.matmul(
                        ps,
                        xT_sb[:, ko, st * P:(st + 1) * P],
                        wv_sb[:, ko, :],
                        start=(ko == 0),
                        stop=(ko == NKO - 1),
                    )
                nc.vector.tensor_copy(
                    out=v_aug[:, st, 0:130].rearrange("p (h x) -> p h x", x=65)[:, :, 1:65],
                    in_=ps.rearrange("p (h x) -> p h x", x=64),
                )

            def attn_unit(b, qb):
                # both local heads; score matmuls on PE row-groups 0-1 / 2-3
                # run concurrently; k-tiles in pairs -> [128, 2, 512] exp ops
                numer = [
                    psSmall.tile([65, 512], f32, tag="small", name="ps_nm_t")
                    for _ in range(2)
                ]
                nkt = 4 * qb + 4
                for kt0 in range(0, nkt, 2):
                    sc = [
                        psBig.tile([P, 2, 512], f32, tag="big", name="ps_sc_t")
                        for _ in range(2)
                    ]
                    ex = [
                        sba.tile([P, 2, 512], bf16, tag=f"exp{hl}", name="sb_ex_t")
                        for hl in range(2)
                    ]
                    for j in range(2):
                        for hl in range(2):
                            nc.tensor.matmul(
                                sc[hl][:, j, :],
                                kT_sb[64 * hl:64 * hl + 64,
                                      S * b + (kt0 + j) * P:S * b + (kt0 + j + 1) * P],
                                qT_sb[64 * hl:64 * hl + 64,
                                      S * b + qb * 512:S * b + (qb + 1) * 512],
                                start=True,
                                stop=True,
                            )
                    for hl in range(2):
                        nc.scalar.activation(ex[hl], sc[hl], Exp, scale=0.125)
                    r = kt0 - 4 * qb
                    if r >= 0:
                        mrow = masks_sb[:, r * 512:(r + 2) * 512].rearrange(
                            "p (j f) -> p j f", j=2
                        )
                        for hl in range(2):
                            nc.vector.tensor_mul(out=ex[hl], in0=ex[hl], in1=mrow)
                    for j in range(2):
                        kt = kt0 + j
                        for hl in range(2):
                            nc.tensor.matmul(
                                numer[hl],
                                v_aug[:, 16 * b + kt, 65 * hl:65 * hl + 65],
                                ex[hl][:, j, :],
                                start=(kt == 0),
                                stop=(kt == nkt - 1),
                            )
                for hl in range(2):
                    recip = sba.tile([1, 512], f32, tag="recip", name="sb_rc_t")
                    nc.vector.reciprocal_approx_fast(recip, numer[hl][0:1, :])
                    rb = sba.tile([65, 512], f32, tag="rbcast", name="sb_rb_t")
                    nc.gpsimd.partition_broadcast(rb, recip)
                    attn = sba.tile([65, 512], bf16, tag="attn", name="sb_at_t")
                    nc.vector.tensor_mul(out=attn, in0=numer[hl][:, :], in1=rb)
                    nc.sync.dma_start(
                        ag_in[4 * b + qb, 64 * hl:64 * hl + 64, :], attn[1:65, :]
                    )

            def gather_slot(s):
                # fire the per-slot exchange as soon as slot s's attention is
                # written; all but the last overlap remaining compute
                nc.gpsimd.collective_compute(
                    "AllGather",
                    mybir.AluOpType.bypass,
                    replica_groups=[list(range(8))],
                    ins=[ag_in[s].opt()],
                    outs=[ag_out[s].opt()],
                )

            # batch-0 inputs first
            for sb in range(4):
                proj_block(wq_sb, qT_sb, sb)
                proj_block(wk_sb, kT_sb, sb)
            for st in range(16):
                v_block(st)

            # batch-0 attention interleaved with batch-1 projections
            a1 = []
            for sb in range(4, 8):
                a1.append(lambda sb=sb: proj_block(wq_sb, qT_sb, sb))
                a1.append(lambda sb=sb: proj_block(wk_sb, kT_sb, sb))
            for st in range(16, 32):
                a1.append(lambda st=st: v_block(st))
            for qb in range(QB):
                attn_unit(0, qb)
                gather_slot(qb)
                take, a1 = a1[:6], a1[6:]
                for thunk in take:
                    thunk()
            for thunk in a1:
                thunk()
            for qb in range(QB):
                attn_unit(1, qb)
                gather_slot(4 + qb)

            # ---- phase D: output projection; this core's slice selected by
            # a partition-id-offset DMA out of its slot's gather ----
            pid = nc.sync.partition_id()
            attn_all = const.tile([P, 8, 512], bf16, name="attn_all")
            for ci in range(8):
                nc.sync.dma_start(
                    attn_all[:, ci, :],
                    ag_out[bass_ds(pid, 1), ci].rearrange("o p f -> (o p) f"),
                )
            out_sb = const.tile([P, NKO, 512], f32, name="out_sb")
            outT_r = outT.rearrange("(mo p) f -> p mo f", p=P)
            for mo in range(NKO):
                ps = psBig.tile([P, 2, 512], f32, tag="big", name="ps_out")
                for ci in range(8):
                    nc.tensor.matmul(
                        ps[:, 0, :],
                        wo_sb[:, ci, mo * P:(mo + 1) * P],
                        attn_all[:, ci, :],
                        start=(ci == 0),
                        stop=(ci == 7),
                    )
                nc.scalar.activation(
                    out_sb[:, mo, :], ps[:, 0, :], Identity,
                    bias=bo_sb[:, mo:mo + 1], scale=1.0,
                )
                nc.sync.dma_start(outT_r[:, mo:mo + 1, :], out_sb[:, mo:mo + 1, :])

    nc.compile()
    _built = nc
    return nc


def _host_masks():
    p = np.arange(P)[:, None]
    f = np.arange(512)[None, :]
    m = np.zeros((P, 4, 512), np.float32)
    for r in range(4):
        m[:, r, :] = (f >= P * r + p).astype(np.float32)
    return np.ascontiguousarray(m.reshape(P, 2048)).astype(BF16)


def kernel(**inputs):
    global LAST_RESULTS
    from concourse import bass_utils

    x = np.asarray(inputs["x"], np.float32)
    W_q = np.asarray(inputs["W_q"], np.float32)
    W_k = np.asarray(inputs["W_k"], np.float32)
    W_v = np.asarray(inputs["W_v"], np.float32)
    W_o = np.asarray(inputs["W_o"], np.float32)
    b_o = np.asarray(inputs["b_o"], np.float32)

    nc = _build()

    xT_all = np.ascontiguousarray(
        np.concatenate([x[0].T, x[1].T], axis=1)
    ).astype(BF16)
    wo_b = np.ascontiguousarray(W_o).astype(BF16)
    bo_t = np.ascontiguousarray(b_o.reshape(NKO, P).T).astype(np.float32)
    masks = _host_masks()

    in_maps = []
    for c in range(8):
        sl = slice(P * c, P * (c + 1))
        in_maps.append({
            "xT": xT_all,
            "wq": np.ascontiguousarray(W_q[:, sl]).astype(BF16),
            "wk": np.ascontiguousarray(W_k[:, sl]).astype(BF16),
            "wv": np.ascontiguousarray(W_v[:, sl]).astype(BF16),
            "wo": wo_b,
            "bo": bo_t,
            "masks": masks,
        })

    res = bass_utils.run_bass_kernel_spmd(nc, in_maps, core_ids=list(range(8)))
    LAST_RESULTS = res

    out = np.empty((B, S, E), np.float32)
    for c in range(8):
        b, qb = c // 4, c % 4
        out[b, 512 * qb:512 * (qb + 1), :] = np.asarray(
            res.results[c]["outT"], np.float32
        ).T
    return out
